# revision 1
# baseline (speedup 1.0000x reference)
"""GQA causal-attention prefill kernel for Trainium2 (8 NeuronCores), v3.

Sharding: head-parallel - core c gets query heads 4c..4c+3 and kv head c.

Per-core device algorithm (all matmuls bf16, scores transposed [key, q]):
  S^T[key, q] = kT_blk.T @ qT_blk                  (PE, exact causal widths)
  P^T = exp(SCALE * S^T)  via either
        - ACT real exp (diag groups + some non-diag groups)
        - DVE Schraudolph bit-trick exp: i16 = rint(S*A + B) bitcast bf16
  diag-block causal triangles zeroed by mask multiply (Pool)
  outT[d, q] += V_blk.T @ P^T_blk                  (PE, PSUM accumulate)
  pair[key, q] = P^T_blk0 + P^T_blk1 into "ship" tile slots (DVE/Pool)
  out-copy PSUM->SBUF bf16                         (ACT)
  DMA out: unnormalized outT (bf16) + per-(h,M) ship tile of pair sums

Host: l[q] = sum over all shipped pair rows (f32); out = outT / l.
No l-matmul on PE, no fold adds, no on-device normalization.
"""

import numpy as np
import ml_dtypes

BF16 = ml_dtypes.bfloat16

SEQ = 2048
NUM_HEADS = 32
NUM_KV_HEADS = 8
D = 128
NCORES = 8
HPC = NUM_HEADS // NCORES
SCALE = float(1.0 / np.sqrt(D))

P = 128
QB = 512
NQB = SEQ // QB
NKB = SEQ // P
NSLOT = 2 * NQB  # max pair slots per (h, M): 2M+2 <= 8

A_S = float(SCALE * 128.0 / np.log(2.0))
B_S = 16255.0

_COMPILED = {}


def _groups():
    gs = []
    for h in range(HPC):
        for M in range(NQB):
            npairs = 2 * (M + 1)
            for gp in range(npairs):
                gs.append((h, M, gp, gp == 0, gp == npairs - 1))
    return gs


def _exp_engine(nd_index):
    # 20 of 48 non-diag groups on ACT ("A"), rest DVE Schraudolph ("V")
    pat = "AVVAVVAAVVAV"  # 19/48 on ACT
    return pat[nd_index % len(pat)]


def _build(num_devices=NCORES, reps=1):
    import concourse.mybir as mybir
    import concourse.tile as tile
    from concourse import bacc

    f32 = mybir.dt.float32
    bf16 = mybir.dt.bfloat16
    i16 = mybir.dt.int16
    Exp = mybir.ActivationFunctionType.Exp
    Copy = mybir.ActivationFunctionType.Copy
    MULT = mybir.AluOpType.mult
    ADD = mybir.AluOpType.add

    nc = bacc.Bacc(
        "TRN2", target_bir_lowering=False, debug=False, num_devices=num_devices
    )

    qT_d = nc.dram_tensor("qT", [HPC, P, SEQ], bf16, kind="ExternalInput")
    kT_d = nc.dram_tensor("kT", [P, SEQ], bf16, kind="ExternalInput")
    v_d = nc.dram_tensor("v", [P, SEQ], bf16, kind="ExternalInput")
    mask_d = nc.dram_tensor("mask", [P, P], bf16, kind="ExternalInput")
    outT_d = nc.dram_tensor("outT", [HPC, NQB, P, QB], bf16, kind="ExternalOutput")
    # ship layout: [h, M, partition(key), slot, q] so SBUF [p, slot, q] maps 1:1
    acc_d = nc.dram_tensor("acc", [HPC, NQB, P, NSLOT, QB], bf16,
                           kind="ExternalOutput")

    with tile.TileContext(nc) as tc:
        with (
            tc.tile_pool(name="const", bufs=1) as cpool,
            tc.tile_pool(name="pt", bufs=10, space="SBUF") as ptpool,
            tc.tile_pool(name="ship", bufs=2) as shpool,
            tc.tile_pool(name="ob", bufs=3) as obpool,
            tc.tile_pool(name="st", bufs=3, space="PSUM") as stpool,
            tc.tile_pool(name="acc", bufs=2, space="PSUM") as accpool,
        ):
            kT_sb = [
                cpool.tile([P, QB], bf16, tag=f"kT{i}", name=f"kT_sb{i}")
                for i in range(4)
            ]
            v_sb = [
                cpool.tile([P, QB], bf16, tag=f"v{i}", name=f"v_sb{i}")
                for i in range(4)
            ]
            q_sb = [
                [
                    cpool.tile([P, QB], bf16, tag=f"q{h}_{m}", name=f"q_sb{h}_{m}")
                    for m in range(NQB)
                ]
                for h in range(HPC)
            ]
            mask_sb = cpool.tile([P, P], bf16, tag="mask")
            warm_sb = cpool.tile([P, 1], f32, tag="warm")

            nc.sync.dma_start(kT_sb[0][:], kT_d.ap()[:, 0:QB])
            nc.vector.memset(warm_sb[:], 0.0)
            nc.scalar.activation(warm_sb[:], warm_sb[:], Exp, scale=SCALE)
            nc.scalar.dma_start(q_sb[0][0][:], qT_d.ap()[0][:, 0:QB])
            nc.sync.dma_start(mask_sb[:], mask_d.ap())
            nc.sync.dma_start(v_sb[0][:], v_d.ap()[:, 0:QB])
            nc.sync.dma_start(q_sb[0][1][:], qT_d.ap()[0][:, QB : 2 * QB])
            nc.sync.dma_start(kT_sb[1][:], kT_d.ap()[:, QB : 2 * QB])
            nc.sync.dma_start(v_sb[1][:], v_d.ap()[:, QB : 2 * QB])
            nc.sync.dma_start(q_sb[0][2][:], qT_d.ap()[0][:, 2 * QB : 3 * QB])
            nc.sync.dma_start(kT_sb[2][:], kT_d.ap()[:, 2 * QB : 3 * QB])
            nc.sync.dma_start(v_sb[2][:], v_d.ap()[:, 2 * QB : 3 * QB])
            nc.sync.dma_start(q_sb[0][3][:], qT_d.ap()[0][:, 3 * QB : 4 * QB])
            nc.sync.dma_start(kT_sb[3][:], kT_d.ap()[:, 3 * QB : 4 * QB])
            nc.sync.dma_start(v_sb[3][:], v_d.ap()[:, 3 * QB : 4 * QB])
            for h in range(1, HPC):
                for m in range(NQB):
                    nc.sync.dma_start(
                        q_sb[h][m][:], qT_d.ap()[h][:, m * QB : (m + 1) * QB]
                    )

            def kT_blk(j):
                return kT_sb[j // 4][:, (j % 4) * P : (j % 4 + 1) * P]

            def v_blk(j):
                return v_sb[j // 4][:, (j % 4) * P : (j % 4 + 1) * P]

            groups = _groups()
            nd_idx = {}
            ndc = 0
            for i, (h, M, gp, _, _) in enumerate(groups):
                if gp < 2 * M:
                    nd_idx[i] = ndc
                    ndc += 1

            state = {}

            def produce(idx):
                h, M, gp, _, _ = groups[idx]
                rep = state.get("rep", 0)
                st = stpool.tile([P, 2, QB], f32, tag="st", name=f"st{rep}_{idx}")
                for t in range(2):
                    j = 2 * gp + t
                    u = j - 4 * M
                    lo = u * P if u > 0 else 0
                    nc.tensor.matmul(
                        st[:, t, lo:QB],
                        lhsT=kT_blk(j),
                        rhs=q_sb[h][M][:, lo:QB],
                        start=True,
                        stop=True,
                    )
                pt = ptpool.tile([P, 2, QB], bf16, tag="pt", name=f"pt{rep}_{idx}")
                if gp == 2 * M:
                    nc.scalar.activation(pt[:], st[:], Exp, scale=SCALE)
                elif gp == 2 * M + 1:
                    nc.scalar.activation(
                        pt[:, :, 2 * P :], st[:, :, 2 * P :], Exp, scale=SCALE
                    )
                else:
                    if _exp_engine(nd_idx[idx]) == "A":
                        nc.scalar.activation(pt[:], st[:], Exp, scale=SCALE)
                    else:
                        nc.vector.tensor_scalar(
                            pt[:].bitcast(i16), st[:], A_S, B_S, op0=MULT, op1=ADD
                        )
                if gp >= 2 * M:
                    for t in range(2):
                        u = 2 * gp + t - 4 * M
                        w = pt[:, t, u * P : (u + 1) * P]
                        nc.gpsimd.tensor_tensor(w, w, mask_sb[:], op=MULT)
                state[idx] = pt

            def consume(idx):
                h, M, gp, first, last = groups[idx]
                rep = state.get("rep", 0)
                pt = state.pop(idx)
                if first:
                    state["out_ps", h, M] = accpool.tile(
                        [P, QB], f32, tag="out", name=f"out{rep}_{h}_{M}"
                    )
                    state["ship", h, M] = shpool.tile(
                        [P, 2 * M + 2, QB], bf16, tag=f"ship{M}", name=f"sh{rep}_{h}_{M}"
                    )
                out_ps = state["out_ps", h, M]
                ship = state["ship", h, M]
                for t in range(2):
                    j = 2 * gp + t
                    u = j - 4 * M
                    lo = u * P if u > 0 else 0
                    nc.tensor.matmul(
                        out_ps[:, lo:QB],
                        lhsT=v_blk(j),
                        rhs=pt[:, t, lo:QB],
                        start=(first and t == 0),
                        stop=(last and t == 1),
                    )
                slot = gp  # slot gp for pairs; slot M+1 is pair B (gp=2M+1)
                if gp == 2 * M:
                    # pair A -> slot 2M: [0,P) only block0; add the rest (DVE)
                    nc.gpsimd.tensor_scalar(
                        ship[:, slot, 0:P], pt[:, 0, 0:P], 1.0, 0.0,
                        op0=MULT, op1=ADD,
                    )
                    nc.vector.tensor_add(
                        ship[:, slot, P:QB], pt[:, 0, P:QB], pt[:, 1, P:QB]
                    )
                elif gp == 2 * M + 1:
                    # pair B -> slot 2M+1, live [2P, QB) (Pool); host ignores
                    # cols [0, 2P) of this slot
                    nc.gpsimd.tensor_scalar(
                        ship[:, slot, 2 * P : 3 * P], pt[:, 0, 2 * P : 3 * P],
                        1.0, 0.0, op0=MULT, op1=ADD,
                    )
                    nc.gpsimd.tensor_tensor(
                        ship[:, slot, 3 * P : QB],
                        pt[:, 0, 3 * P : QB],
                        pt[:, 1, 3 * P : QB],
                        op=ADD,
                    )
                else:
                    nc.vector.tensor_add(ship[:, slot], pt[:, 0], pt[:, 1])
                    if M == 3 and gp == 3:
                        nc.sync.dma_start(
                            acc_d.ap()[h][M][:, 0:4, :], ship[:, 0:4, :]
                        )
                    elif M == 3 and gp == 5:
                        nc.sync.dma_start(
                            acc_d.ap()[h][M][:, 4:6, :], ship[:, 4:6, :]
                        )
                    elif M != 3 and gp == 2 * M - 1:
                        # bulk ship: all non-diag slots [0, 2M) complete
                        nc.sync.dma_start(
                            acc_d.ap()[h][M][:, 0 : 2 * M, :],
                            ship[:, 0 : 2 * M, :],
                        )

                if last:
                    nc.sync.dma_start(
                        acc_d.ap()[h][M][:, 2 * M : 2 * M + 2, :],
                        ship[:, 2 * M : 2 * M + 2, :],
                    )
                    osb = obpool.tile([P, QB], bf16, tag="ob", name=f"ob{rep}_{h}{M}")
                    if True:
                        nc.scalar.activation(
                            osb[:, 0 : 2 * P], out_ps[:, 0 : 2 * P], Copy
                        )
                        nc.sync.dma_start(
                            outT_d.ap()[h][M][:, 0 : 2 * P], osb[:, 0 : 2 * P]
                        )
                        nc.scalar.activation(osb[:, 2 * P :], out_ps[:, 2 * P :], Copy)
                        nc.sync.dma_start(
                            outT_d.ap()[h][M][:, 2 * P :], osb[:, 2 * P :]
                        )
                    else:
                        nc.scalar.activation(osb[:], out_ps[:], Copy)
                        nc.sync.dma_start(outT_d.ap()[h][M], osb[:])
                    del state["out_ps", h, M]
                    del state["ship", h, M]

            LOOKAHEAD = 8
            for rep in range(reps):
                state["rep"] = rep
                for i in range(min(LOOKAHEAD, len(groups))):
                    produce(i)
                for i in range(len(groups)):
                    if i + LOOKAHEAD < len(groups):
                        produce(i + LOOKAHEAD)
                    consume(i)

    nc.compile()
    return nc


def _host_mask():
    p = np.arange(P)[:, None]
    c = np.arange(P)[None, :]
    return (c >= p).astype(BF16)


def kernel(q, k, v, k_cache=None, v_cache=None, slot_mapping=None, **_):
    from concourse.bass_utils import run_bass_kernel_spmd

    if "nc" not in _COMPILED:
        _COMPILED["nc"] = _build()
    nc = _COMPILED["nc"]

    q = np.asarray(q, dtype=np.float32)
    k = np.asarray(k, dtype=np.float32)
    v = np.asarray(v, dtype=np.float32)

    mask = _host_mask()
    in_maps = []
    for c in range(NCORES):
        qT_c = np.ascontiguousarray(
            q[:, HPC * c : HPC * (c + 1), :].transpose(1, 2, 0)
        ).astype(BF16)
        kT_c = np.ascontiguousarray(k[:, c, :].T).astype(BF16)
        v_c = np.ascontiguousarray(
            v[:, c, :].reshape(NKB, P, D).transpose(1, 0, 2).reshape(P, SEQ)
        ).astype(BF16)
        in_maps.append({"qT": qT_c, "kT": kT_c, "v": v_c, "mask": mask})

    res = run_bass_kernel_spmd(nc, in_maps, list(range(NCORES)))

    out = np.empty((SEQ, NUM_HEADS, D), np.float32)
    for c in range(NCORES):
        oT = res.results[c]["outT"].astype(np.float32)   # [HPC, NQB, d, q]
        ac = res.results[c]["acc"]                        # [HPC, NQB, p, slot, q]
        for h in range(HPC):
            for M in range(NQB):
                a = ac[h, M].astype(np.float32)  # [128, NSLOT, 512]
                l = a[:, 0 : 2 * M + 1, :].sum(axis=(0, 1))  # pairs + pair A
                l[2 * P :] += a[:, 2 * M + 1, 2 * P :].sum(axis=0)  # pair B
                out[M * QB : (M + 1) * QB, HPC * c + h, :] = (oT[h, M] / l).T
    return out

